# revision 53
# baseline (speedup 1.0000x reference)
"""CaptioningRNN (LSTM + tiny spatial attention) Trainium2 kernel.

Contract: kernel(**inputs) takes FULL inputs (numpy), returns FULL output
(N, T, H) float32.  Internally: data-parallel over batch N across 8
NeuronCores (16 sequences per core, zero cross-core traffic).

Per-core algorithm (v8):
  init : G[n] = Aflat[n]^T @ Wattn  precomputed on the PE (rank M=16
         low-rank structure of the attention), stored block-packed in
         SBUF as G_sb[16*n'+m, c, j] for seq chunks c of 8.
  loop : 512 sequential LSTM steps.  Gate banks are ordered f|o|i|g and
         the g columns are host-scaled by 2 so one fused strided ACT op
         computes tanh(psa/2) for all four gates.  The cell update runs
         in TRANSPOSED layout (h on partitions): gates are PE-transposed
         right after the tanh, the c/h elementwise chain runs at full
         128-lane width, and h comes out already transposed for the
         next step's matmuls.  Step t-1's tail chunks are interleaved
         with step t's Wh groups; P = x@Wx is produced into an SBUF
         ring (2 matmuls/step) and injected via selector matmuls.
         PSUM gates rotate partitions 0/32/64 (tile_position) so three
         steps share the same 4 banks.
"""

import sys
import numpy as np

sys.path.insert(0, "/opt/trn_rl_repo")

import ml_dtypes

BF16 = ml_dtypes.bfloat16

N, T, D, H, M = 128, 512, 512, 512, 16
NCORES = 8
NL = N // NCORES          # 16 sequences per core
KC = 4                    # 512 = 4 chunks of 128 (contraction dims)
J = 4 * H                 # 2048 gate columns
TB = 8                    # time steps per P row block
NCH = 2                   # seq chunks of 8 for the G trick

_CACHE = {}


def build(t_steps=T, has_bias=False):
    from concourse import bacc, mybir
    import concourse.tile as tile

    f32 = mybir.dt.float32
    bf16 = mybir.dt.bfloat16
    mult = mybir.AluOpType.mult
    add = mybir.AluOpType.add
    AF = mybir.ActivationFunctionType
    AX = mybir.AxisListType.X

    nblk = (t_steps + TB - 1) // TB

    nc = bacc.Bacc("TRN2", target_bir_lowering=False, debug=False,
                   num_devices=NCORES)

    # ---- I/O -----------------------------------------------------------
    xs = nc.dram_tensor("xs", [nblk, 128, KC, 128], bf16, kind="ExternalInput")
    at_d = nc.dram_tensor("at", [128, KC, NL, M], bf16, kind="ExternalInput")
    wx_d = nc.dram_tensor("wx", [128, KC, J], bf16, kind="ExternalInput")
    wh_d = nc.dram_tensor("wh", [128, KC, J], bf16, kind="ExternalInput")
    wa_d = nc.dram_tensor("wa", [128, KC, J], bf16, kind="ExternalInput")
    h0_d = nc.dram_tensor("h0t", [128, KC, NL], bf16, kind="ExternalInput")
    c0_d = nc.dram_tensor("c0t", [128, KC, NL], f32, kind="ExternalInput")
    id_d = nc.dram_tensor("ident", [NL, NL], bf16, kind="ExternalInput")
    idb_d = nc.dram_tensor("identb", [128, 128], bf16, kind="ExternalInput")
    idf_d = nc.dram_tensor("identf", [NL, NL], f32, kind="ExternalInput")
    oc_d = nc.dram_tensor("ones_col", [128, 1], bf16, kind="ExternalInput")
    or_d = nc.dram_tensor("ones_row", [1, 128], bf16, kind="ExternalInput")
    sel_d = nc.dram_tensor("sel", [128, TB, NL], bf16, kind="ExternalInput")
    msk_d = nc.dram_tensor("mask3", [128, NCH, NL], bf16,
                           kind="ExternalInput")
    if has_bias:
        b_d = nc.dram_tensor("bvec", [1, J], f32, kind="ExternalInput")
    out_d = nc.dram_tensor("out", [NL, t_steps, H], bf16,
                           kind="ExternalOutput")

    inv_sqrt_h = float(1.0 / np.sqrt(H))

    from contextlib import ExitStack
    with tile.TileContext(nc) as tc, ExitStack() as stack:
        # ---- persistent constants -------------------------------------
        cpool = stack.enter_context(tc.tile_pool(name="consts", bufs=1))
        wh_s = cpool.tile([128, KC, J], bf16, name="wh_s")
        wx_s = cpool.tile([128, KC, J], bf16, name="wx_s")
        at_s = cpool.tile([128, KC, NL, M], bf16, name="at_s")
        g_sb = cpool.tile([128, NCH, J], bf16, name="g_sb")
        h0_s = cpool.tile([128, KC, NL], bf16, name="h0_s")
        c0_s = cpool.tile([128, KC, NL], f32, name="c0_s")
        id_s = cpool.tile([NL, NL], bf16, name="id_s")
        idb_s = cpool.tile([128, 128], bf16, name="idb_s")
        idf_s = cpool.tile([NL, NL], f32, name="idf_s")
        oc_s = cpool.tile([128, 1], bf16, name="oc_s")
        or_s = cpool.tile([1, 128], bf16, name="or_s")
        sel_s = cpool.tile([128, TB, NL], bf16, name="sel_s")
        msk_s = cpool.tile([128, NCH, NL], bf16, name="msk_s")
        nc.sync.dma_start(out=wh_s[:, :, :], in_=wh_d.ap()[:, :, :])
        nc.sync.dma_start(out=wx_s[:, :, :], in_=wx_d.ap()[:, :, :])
        nc.sync.dma_start(out=at_s[:, :, :, :], in_=at_d.ap()[:, :, :, :])
        nc.sync.dma_start(out=h0_s[:, :, :], in_=h0_d.ap()[:, :, :])
        nc.sync.dma_start(out=c0_s[:, :, :], in_=c0_d.ap()[:, :, :])
        nc.sync.dma_start(out=id_s[:, :], in_=id_d.ap()[:, :])
        nc.sync.dma_start(out=idb_s[:, :], in_=idb_d.ap()[:, :])
        nc.sync.dma_start(out=idf_s[:, :], in_=idf_d.ap()[:, :])
        nc.sync.dma_start(out=oc_s[:, :], in_=oc_d.ap()[:, :])
        nc.sync.dma_start(out=or_s[:, :], in_=or_d.ap()[:, :])
        nc.sync.dma_start(out=sel_s[:, :, :], in_=sel_d.ap()[:, :, :])
        nc.sync.dma_start(out=msk_s[:, :, :], in_=msk_d.ap()[:, :, :])

        # pools that live for the whole kernel
        ppool = stack.enter_context(tc.tile_pool(name="pring", bufs=3))
        xpool = stack.enter_context(tc.tile_pool(name="xring", bufs=3))
        stp = stack.enter_context(tc.tile_pool(name="state", bufs=2))
        wk = stack.enter_context(tc.tile_pool(name="work", bufs=2))
        hop = stack.enter_context(tc.tile_pool(name="hout", bufs=3))
        psa_p = stack.enter_context(tc.tile_pool(name="ps_a", bufs=1,
                                                 space="PSUM"))
        pp_ps = stack.enter_context(tc.tile_pool(name="ps_p", bufs=1,
                                                 space="PSUM"))
        att_ps = stack.enter_context(tc.tile_pool(name="ps_t", bufs=1,
                                                  space="PSUM"))
        unt_ps = stack.enter_context(tc.tile_pool(name="ps_u", bufs=1,
                                                  space="PSUM"))

        # ---- init: G = blockpack(Aflat^T @ Wattn) ---------------------
        with tc.tile_pool(name="wa_tmp", bufs=1) as wap:
            wa_s = wap.tile([128, KC, J], bf16, name="wa_s")
            nc.sync.dma_start(out=wa_s[:, :, :], in_=wa_d.ap()[:, :, :])
            if has_bias:
                bf_s = cpool.tile([1, J], f32, name="bf_s")
                nc.sync.dma_start(out=bf_s[:, :], in_=b_d.ap()[:, :])
                bb_s = cpool.tile([1, J], bf16, name="bb_s")
                nc.vector.tensor_copy(bb_s[:, :], bf_s[:, :])
            for c in range(NCH):
                for jt in range(4):
                    jsl = slice(jt * 512, (jt + 1) * 512)
                    gps = pp_ps.tile([128, 512], f32, tag="pps", name="gps")
                    for kc in range(KC):
                        nc.tensor.matmul(
                            gps[:, :],
                            at_s[:, kc, c * 8:(c + 1) * 8, :]
                            .rearrange("p a b -> p (a b)"),
                            wa_s[:, kc, jsl],
                            start=(kc == 0), stop=(kc == KC - 1))
                    nc.vector.tensor_copy(g_sb[:, c, jsl], gps[:, :])

        # ---- persistent PSUM ------------------------------------------
        # gates: four per-bank tiles (f|o|i|g); step t writes
        # partitions 32*(t%3):+16 of each
        psaT = [psa_p.tile([128, 512], f32, tag=f"a{jt}", name="psaT")
                for jt in range(4)]
        # att bank: psz (scores colsum) alone — it holds a
        # multi-instruction accumulation group each step
        attb = att_ps.tile([128, 256], f32, tag="t", name="attb")
        psz = attb[0:1, 0:256].rearrange("p (a b) -> p a b", a=NL)
        # untranspose bank: h back in n-layout for output DMA +
        # transposed gates (all atomic writes)
        untb = unt_ps.tile([128, 776], bf16, tag="u", name="untb")
        untr = untb[0:NL, 0:512]
        pstg = untb[:, 520:776].rearrange("p (g k n) -> p g k n", g=4, k=KC)
        # softmax weight transposes get their own bank so they never
        # wait on the output flush
        pwb = unt_ps.tile([128, NCH, 2], bf16, tag="w", name="pwb")
        pstw = pwb

        pblks = {}
        xts = {}
        prod_pps = {}

        def stage_x(b):
            xt = xpool.tile([128, KC, 128], bf16, tag="xt", name="xt")
            nc.sync.dma_start(out=xt[:, :, :], in_=xs.ap()[b, :, :, :])
            xts[b] = xt

        def produce_half(b, hh, defer=False):
            """2 of the 16 matmuls of P[b] = x_block @ Wx (+b); hh=0..7.
            With defer=True the PSUM->SBUF copy is returned as a thunk so
            it can be emitted late (it has ~7 steps of slack and must not
            sit ahead of the tail's chain-critical reads in the DVE
            queue)."""
            if hh == 0:
                pblks[b] = ppool.tile([128, J], bf16, tag="pblk",
                                      name="pblk")
            jt, kp = divmod(hh, 2)
            jsl = slice(jt * 512, (jt + 1) * 512)
            if kp == 0:
                prod_pps[b] = pp_ps.tile([128, 512], f32, tag="pps",
                                         name="pps")
            pps = prod_pps[b]
            for k2 in range(2):
                kc = 2 * kp + k2
                nc.tensor.matmul(pps[:, :], xts[b][:, kc, :],
                                 wx_s[:, kc, jsl],
                                 start=(kc == 0),
                                 stop=(kc == KC - 1) and not has_bias)
            if kp == 1:
                if has_bias:
                    nc.tensor.matmul(pps[:, :], or_s[:, :], bb_s[:, jsl],
                                     start=False, stop=True)

                def copy():
                    nc.vector.tensor_copy(pblks[b][:, jsl], pps[:, :])
                if defer:
                    return copy
                copy()
            return None

        # prime the ring
        for b in range(min(2, nblk)):
            stage_x(b)
            for hh in range(2 * KC):
                produce_half(b, hh)

        cT = c0_s
        hT = h0_s
        prev = None          # (psa slice, out-row) of step t-1
        flush_row = None     # output row whose h sits in untr, not yet sent

        def tail_reads(ppsa):
            """Copy the previous step's gates out of PSUM in four
            parallel pieces (DVE + gpsimd) — the only reads of the psa
            banks, so the next step's writes unblock early."""
            rs = []
            for jt in range(4):
                r = wk.tile([NL, 512], bf16, tag=f"r{jt}", name="r")
                if jt % 2 == 0:
                    nc.vector.tensor_copy(r[:, :], ppsa[jt][:, :])
                else:
                    nc.scalar.copy(r[:, :], ppsa[jt][:, :])
                rs.append(r)
            return rs

        def tail_tr(rs, kc):
            """PE-transpose the raw gates of kc into pstg."""
            for g4 in range(4):
                nc.tensor.transpose(pstg[:, g4, kc, :],
                                    rs[g4][:, kc * 128:(kc + 1) * 128],
                                    id_s[:, :])

        def tail_chunk(ch, cT_old, cT_new, hT_new):
            """Cell update for kc pair ch in transposed layout:
            tanh on [128, ...]-shaped ops, then the c/h chain at full
            lane width.  h lands in hT_new and (via untr) the output."""
            ksl = slice(2 * ch, 2 * ch + 2)
            tfT = wk.tile([128, 4, 2, NL], bf16, tag=f"tf{ch}", name="tfT")
            nc.scalar.activation(tfT[:, :, :, :], pstg[:, :, ksl, :],
                                 AF.Tanh, scale=0.5)
            sfoi = wk.tile([128, 3, 2, NL], bf16, tag=f"sf{ch}", name="sfoi")
            nc.vector.tensor_scalar(sfoi[:, :, :, :], tfT[:, 0:3, :, :],
                                    0.5, 0.5, mult, add)
            t1 = wk.tile([128, 2, NL], f32, tag=f"t1{ch}", name="t1")
            nc.vector.tensor_tensor(t1[:, :, :], sfoi[:, 0, :, :],
                                    cT_old[:, ksl, :], mult)
            t2 = wk.tile([128, 2, NL], bf16, tag=f"t2{ch}", name="t2")
            nc.vector.tensor_tensor(t2[:, :, :], sfoi[:, 2, :, :],
                                    tfT[:, 3, :, :], mult)
            nc.vector.tensor_tensor(cT_new[:, ksl, :], t1[:, :, :],
                                    t2[:, :, :], add)
            tc_t = wk.tile([128, 2, NL], bf16, tag=f"tc{ch}", name="tc_t")
            nc.scalar.activation(tc_t[:, :, :], cT_new[:, ksl, :], AF.Tanh)
            nc.vector.tensor_tensor(hT_new[:, ksl, :], sfoi[:, 1, :, :],
                                    tc_t[:, :, :], mult)

        def untr_kc(hT_new, kc):
            nc.tensor.transpose(untr[:, kc * 128:(kc + 1) * 128],
                                hT_new[:, kc, :], idb_s[:, :])

        def flush_out(row):
            h_f = hop.tile([NL, H], bf16, tag="hf", name="h_f")
            nc.scalar.copy(h_f[:, :], untr[:, :])
            nc.sync.dma_start(out=out_d.ap()[:, row, :], in_=h_f[:, :])

        for t in range(t_steps):
            b_i, tt = divmod(t, TB)
            pblk = pblks[b_i]
            par = t % 3
            psl = slice(32 * par, 32 * par + NL)
            psa = [psaT[jt][psl, :] for jt in range(4)]

            # -- P production for block b+2 fills the step-boundary gap
            nb = b_i + 2
            prod_copy = None
            if nb < nblk:
                if tt == 0:
                    stage_x(nb)
                prod_copy = produce_half(nb, tt, defer=True)

            s2 = wk.tile([128, KC, NL, M], bf16, tag="s2", name="s2")
            if prev is not None:
                cT_new = stp.tile([128, KC, NL], f32, tag="c", name="cT_n")
                hT_new = stp.tile([128, KC, NL], bf16, tag="h", name="hT_n")
                rs = tail_reads(prev[0])
                if flush_row is not None:
                    flush_out(flush_row)
                tail_tr(rs, 0)
                tail_tr(rs, 1)
                # P inject opens the psum group; unblocks once the four
                # reads above retire
                for jt in range(4):
                    nc.tensor.matmul(psa[jt], sel_s[:, tt, :],
                                     pblk[:, jt * 512:(jt + 1) * 512],
                                     start=True, stop=False,
                                     skip_group_check=True)
                tail_tr(rs, 2)
                tail_tr(rs, 3)
                tail_chunk(0, cT, cT_new, hT_new)
                for kc in (0, 1):
                    nc.gpsimd.tensor_tensor(
                        s2[:, kc, :, :], at_s[:, kc, :, :],
                        hT_new[:, kc, :, None].broadcast_to([128, NL, M]),
                        mult)
                for jt in range(4):
                    nc.tensor.matmul(psa[jt], hT_new[:, 0, :],
                                     wh_s[:, 0, jt * 512:(jt + 1) * 512],
                                     start=False, stop=False,
                                     skip_group_check=True)
                untr_kc(hT_new, 0)
                untr_kc(hT_new, 1)
                tail_chunk(1, cT, cT_new, hT_new)
                for kc in (2, 3):
                    nc.gpsimd.tensor_tensor(
                        s2[:, kc, :, :], at_s[:, kc, :, :],
                        hT_new[:, kc, :, None].broadcast_to([128, NL, M]),
                        mult)
                for jt in range(4):
                    nc.tensor.matmul(psa[jt], hT_new[:, 1, :],
                                     wh_s[:, 1, jt * 512:(jt + 1) * 512],
                                     start=False, stop=False,
                                     skip_group_check=True)
                for kc in range(KC):
                    nc.tensor.matmul(psz[:, :, :], oc_s[:, :],
                                     s2[:, kc, :, :],
                                     start=(kc == 0), stop=(kc == KC - 1))
                for q in (2, 3):
                    for jt in range(4):
                        nc.tensor.matmul(psa[jt], hT_new[:, q, :],
                                         wh_s[:, q,
                                              jt * 512:(jt + 1) * 512],
                                         start=False, stop=False,
                                         skip_group_check=True)
                untr_kc(hT_new, 2)
                untr_kc(hT_new, 3)
                cT = cT_new
                hT = hT_new
            else:
                for jt in range(4):
                    nc.tensor.matmul(psa[jt], sel_s[:, tt, :],
                                     pblk[:, jt * 512:(jt + 1) * 512],
                                     start=True, stop=False,
                                     skip_group_check=True)
                for kc in range(KC):
                    nc.gpsimd.tensor_tensor(
                        s2[:, kc, :, :], at_s[:, kc, :, :],
                        hT[:, kc, :, None].broadcast_to([128, NL, M]),
                        mult)
                for q in range(KC):
                    for jt in range(4):
                        nc.tensor.matmul(psa[jt], hT[:, q, :],
                                         wh_s[:, q,
                                              jt * 512:(jt + 1) * 512],
                                         start=False, stop=False,
                                         skip_group_check=True)
                for kc in range(KC):
                    nc.tensor.matmul(psz[:, :, :], oc_s[:, :],
                                     s2[:, kc, :, :],
                                     start=(kc == 0), stop=(kc == KC - 1))

            # -- softmax on one partition (overlaps Wh stream on ACT/DVE)
            e_t = wk.tile([1, NL * M], bf16, tag="e", name="e_t")
            nc.scalar.activation(
                e_t[:, :], psz[:, :, :].rearrange("p a b -> p (a b)"),
                AF.Exp, scale=inv_sqrt_h)
            sum_e = wk.tile([1, NL, 1], f32, tag="sume", name="sum_e")
            nc.vector.tensor_reduce(
                sum_e[:, :, :],
                e_t[:, :].rearrange("p (a b) -> p a b", a=NL), AX, add)
            rec = wk.tile([1, NL, 1], f32, tag="rec", name="rec")
            nc.vector.reciprocal(rec[:, :, :], sum_e[:, :, :])
            w_t = wk.tile([1, NL, M], bf16, tag="wt", name="w_t")
            nc.vector.tensor_tensor(
                w_t[:, :, :],
                e_t[:, :].rearrange("p (a b) -> p a b", a=NL),
                rec[:, :, :].broadcast_to([1, NL, M]), mult)

            # -- block-diagonal softmax weights: bd[16n+m, c, q]
            for c in range(NCH):
                nc.tensor.transpose(
                    pstw[:, c, 0:1],
                    w_t[:, :, :].rearrange("p a b -> p (a b)")
                    [:, c * 128:(c + 1) * 128],
                    id_s[:1, :1])
            bd = wk.tile([128, NCH, NL], bf16, tag="bd", name="bd")
            nc.vector.tensor_tensor(
                bd[:, :, :], msk_s[:, :, :],
                pstw[:, :, 0:1].broadcast_to([128, NCH, NL]), mult)

            # -- attention gate contribution via G (K=16 per chunk)
            for jt in range(4):
                for c in range(NCH):
                    nc.tensor.matmul(psa[jt], bd[:, c, :],
                                     g_sb[:, c, jt * 512:(jt + 1) * 512],
                                     start=False, stop=(c == NCH - 1),
                                     skip_group_check=True)

            if prod_copy is not None:
                prod_copy()
            if prev is not None:
                flush_row = prev[1]
            prev = (psa, t)

        # final tail: h(T) for output row T-1
        cT_new = stp.tile([128, KC, NL], f32, tag="c", name="cT_n")
        hT_new = stp.tile([128, KC, NL], bf16, tag="h", name="hT_n")
        rs = tail_reads(prev[0])
        if flush_row is not None:
            flush_out(flush_row)
        for kc in range(KC):
            tail_tr(rs, kc)
        for ch in range(2):
            tail_chunk(ch, cT, cT_new, hT_new)
        for kc in range(KC):
            untr_kc(hT_new, kc)
        flush_out(prev[1])

    nc.compile()
    return nc


def _stage_inputs(x, A, Wx, Wh, Wattn, b, t_steps=T):
    """Shard + lay out inputs per core (host-side numpy staging)."""
    nblk = (t_steps + TB - 1) // TB
    h0 = A.mean(axis=(2, 3)).astype(np.float32)          # (N, H)
    ident = np.eye(NL, dtype=BF16)
    ones_col = np.ones((128, 1), dtype=BF16)
    ones_row = np.ones((1, 128), dtype=BF16)

    # sel[p, tt, q] = 1 iff p == 16*tt + q
    sel = np.zeros((128, TB, NL), dtype=BF16)
    for ttt in range(TB):
        for q in range(NL):
            sel[NL * ttt + q, ttt, q] = 1
    # mask3[16n+m, c, q] = 1 iff q == 8c + n
    mask3 = np.zeros((128, NCH, NL), dtype=BF16)
    for n in range(8):
        for m in range(M):
            for c in range(NCH):
                mask3[M * n + m, c, 8 * c + n] = 1

    # device gate order f|o|i|g (reference is i|f|o|g), and the g block
    # is scaled by 2 so tanh(psa/2) is right for every gate
    gperm = np.concatenate([np.arange(512, 1024), np.arange(1024, 1536),
                            np.arange(0, 512), np.arange(1536, 2048)])
    gscale = np.ones((J,), np.float32)
    gscale[1536:] = 2.0

    def wlay(w):
        w2 = w[:, gperm].astype(np.float32) * gscale
        return np.ascontiguousarray(
            w2.astype(BF16).reshape(KC, 128, J).transpose(1, 0, 2))

    wxs, whs, was = wlay(Wx), wlay(Wh), wlay(Wattn)
    bvec = np.ascontiguousarray(
        (b[gperm].astype(np.float32) * gscale).reshape(1, J))

    def tlay(v, dtype):
        return np.ascontiguousarray(
            v.T.astype(dtype).reshape(KC, 128, NL).transpose(1, 0, 2))

    maps = []
    for k in range(NCORES):
        ns = slice(k * NL, (k + 1) * NL)
        x_sh = x[ns, :t_steps].astype(BF16)              # (NL, t, D)
        # (blk, p, kc, tt*NL+n)
        xT = x_sh.transpose(2, 0, 1).reshape(KC, 128, NL, nblk, TB)
        xs_st = np.ascontiguousarray(
            xT.transpose(3, 1, 0, 4, 2).reshape(nblk, 128, KC, 128))
        A_sh = A[ns].reshape(NL, H, M).astype(BF16)
        at_st = np.ascontiguousarray(
            A_sh.transpose(1, 0, 2).reshape(KC, 128, NL, M)
            .transpose(1, 0, 2, 3))
        h0_sh = h0[ns]                                    # (NL, H)
        m = {
            "xs": xs_st, "at": at_st, "wx": wxs, "wh": whs, "wa": was,
            "h0t": tlay(h0_sh, BF16), "c0t": tlay(h0_sh, np.float32),
            "ident": ident, "identb": np.eye(128, dtype=BF16),
            "identf": np.eye(NL, dtype=np.float32),
            "ones_col": ones_col, "ones_row": ones_row,
            "sel": sel, "mask3": mask3,
        }
        if np.any(b != 0):
            m["bvec"] = bvec
        maps.append(m)
    return maps


def _get_nc(has_bias, t_steps=T):
    key = (has_bias, t_steps)
    if key not in _CACHE:
        _CACHE[key] = build(t_steps=t_steps, has_bias=has_bias)
    return _CACHE[key]


def run_cores(x, A, Wx, Wh, Wattn, b, t_steps=T, trace=False):
    from concourse.bass_utils import run_bass_kernel_spmd
    maps = _stage_inputs(x, A, Wx, Wh, Wattn, b, t_steps=t_steps)
    has_bias = "bvec" in maps[0]
    nc = _get_nc(has_bias, t_steps)
    res = run_bass_kernel_spmd(nc, maps, list(range(NCORES)), trace=trace)
    out = np.concatenate([res.results[k]["out"] for k in range(NCORES)],
                         axis=0)
    return np.asarray(out, dtype=np.float32), res


def kernel(x, A, Wx, Wh, Wattn, b):
    x = np.asarray(x, dtype=np.float32)
    A = np.asarray(A, dtype=np.float32)
    out, _ = run_cores(x, A,
                       np.asarray(Wx, dtype=np.float32),
                       np.asarray(Wh, dtype=np.float32),
                       np.asarray(Wattn, dtype=np.float32),
                       np.asarray(b, dtype=np.float32))
    return out


# revision 55
# speedup vs baseline: 1.4115x; 1.4115x over previous
"""CaptioningRNN (LSTM + tiny spatial attention) Trainium2 kernel.

Contract: kernel(**inputs) takes FULL inputs (numpy), returns FULL output
(N, T, H) float32.  Internally: data-parallel over batch N across 8
NeuronCores (16 sequences per core, zero cross-core traffic).

Per-core algorithm (v8):
  init : G[n] = Aflat[n]^T @ Wattn  precomputed on the PE (rank M=16
         low-rank structure of the attention), stored block-packed in
         SBUF as G_sb[16*n'+m, c, j] for seq chunks c of 8.
  loop : 512 sequential LSTM steps.  Gate banks are ordered f|o|i|g and
         the g columns are host-scaled by 2 so one fused strided ACT op
         computes tanh(psa/2) for all four gates.  The cell update runs
         in TRANSPOSED layout (h on partitions): gates are PE-transposed
         right after the tanh, the c/h elementwise chain runs at full
         128-lane width, and h comes out already transposed for the
         next step's matmuls.  Step t-1's tail chunks are interleaved
         with step t's Wh groups; P = x@Wx is produced into an SBUF
         ring (2 matmuls/step) and injected via selector matmuls.
         PSUM gates rotate partitions 0/32/64 (tile_position) so three
         steps share the same 4 banks.
"""

import sys
import numpy as np

sys.path.insert(0, "/opt/trn_rl_repo")

import ml_dtypes

BF16 = ml_dtypes.bfloat16

N, T, D, H, M = 128, 512, 512, 512, 16
NCORES = 8
NL = N // NCORES          # 16 sequences per core
KC = 4                    # 512 = 4 chunks of 128 (contraction dims)
J = 4 * H                 # 2048 gate columns
TB = 8                    # time steps per P row block
NCH = 2                   # seq chunks of 8 for the G trick

_CACHE = {}


def build(t_steps=T, has_bias=False):
    from concourse import bacc, mybir
    import concourse.tile as tile

    f32 = mybir.dt.float32
    bf16 = mybir.dt.bfloat16
    mult = mybir.AluOpType.mult
    add = mybir.AluOpType.add
    AF = mybir.ActivationFunctionType
    AX = mybir.AxisListType.X

    nblk = (t_steps + TB - 1) // TB

    nc = bacc.Bacc("TRN2", target_bir_lowering=False, debug=False,
                   num_devices=NCORES)

    # ---- I/O -----------------------------------------------------------
    xs = nc.dram_tensor("xs", [nblk, 128, KC, 128], bf16, kind="ExternalInput")
    at_d = nc.dram_tensor("at", [128, KC, NL, M], bf16, kind="ExternalInput")
    wx_d = nc.dram_tensor("wx", [128, KC, J], bf16, kind="ExternalInput")
    wh_d = nc.dram_tensor("wh", [128, KC, J], bf16, kind="ExternalInput")
    wa_d = nc.dram_tensor("wa", [128, KC, J], bf16, kind="ExternalInput")
    h0_d = nc.dram_tensor("h0t", [128, KC, NL], bf16, kind="ExternalInput")
    c0_d = nc.dram_tensor("c0t", [128, KC, NL], f32, kind="ExternalInput")
    id_d = nc.dram_tensor("ident", [NL, NL], bf16, kind="ExternalInput")
    idb_d = nc.dram_tensor("identb", [128, 128], bf16, kind="ExternalInput")
    idf_d = nc.dram_tensor("identf", [NL, NL], f32, kind="ExternalInput")
    oc_d = nc.dram_tensor("ones_col", [128, 1], bf16, kind="ExternalInput")
    or_d = nc.dram_tensor("ones_row", [1, 128], bf16, kind="ExternalInput")
    sel_d = nc.dram_tensor("sel", [128, TB, NL], bf16, kind="ExternalInput")
    msk_d = nc.dram_tensor("mask3", [128, NCH, NL], bf16,
                           kind="ExternalInput")
    if has_bias:
        b_d = nc.dram_tensor("bvec", [1, J], f32, kind="ExternalInput")
    out_d = nc.dram_tensor("out", [NL, t_steps, H], bf16,
                           kind="ExternalOutput")

    inv_sqrt_h = float(1.0 / np.sqrt(H))

    from contextlib import ExitStack
    with tile.TileContext(nc) as tc, ExitStack() as stack:
        # ---- persistent constants -------------------------------------
        cpool = stack.enter_context(tc.tile_pool(name="consts", bufs=1))
        wh_s = cpool.tile([128, KC, J], bf16, name="wh_s")
        wx_s = cpool.tile([128, KC, J], bf16, name="wx_s")
        at_s = cpool.tile([128, KC, NL, M], bf16, name="at_s")
        g_sb = cpool.tile([128, NCH, J], bf16, name="g_sb")
        h0_s = cpool.tile([128, KC, NL], bf16, name="h0_s")
        c0_s = cpool.tile([128, KC, NL], f32, name="c0_s")
        id_s = cpool.tile([NL, NL], bf16, name="id_s")
        idb_s = cpool.tile([128, 128], bf16, name="idb_s")
        idf_s = cpool.tile([NL, NL], f32, name="idf_s")
        oc_s = cpool.tile([128, 1], bf16, name="oc_s")
        or_s = cpool.tile([1, 128], bf16, name="or_s")
        sel_s = cpool.tile([128, TB, NL], bf16, name="sel_s")
        msk_s = cpool.tile([128, NCH, NL], bf16, name="msk_s")
        nc.sync.dma_start(out=wh_s[:, :, :], in_=wh_d.ap()[:, :, :])
        nc.sync.dma_start(out=wx_s[:, :, :], in_=wx_d.ap()[:, :, :])
        nc.sync.dma_start(out=at_s[:, :, :, :], in_=at_d.ap()[:, :, :, :])
        nc.sync.dma_start(out=h0_s[:, :, :], in_=h0_d.ap()[:, :, :])
        nc.sync.dma_start(out=c0_s[:, :, :], in_=c0_d.ap()[:, :, :])
        nc.sync.dma_start(out=id_s[:, :], in_=id_d.ap()[:, :])
        nc.sync.dma_start(out=idb_s[:, :], in_=idb_d.ap()[:, :])
        nc.sync.dma_start(out=idf_s[:, :], in_=idf_d.ap()[:, :])
        nc.sync.dma_start(out=oc_s[:, :], in_=oc_d.ap()[:, :])
        nc.sync.dma_start(out=or_s[:, :], in_=or_d.ap()[:, :])
        nc.sync.dma_start(out=sel_s[:, :, :], in_=sel_d.ap()[:, :, :])
        nc.sync.dma_start(out=msk_s[:, :, :], in_=msk_d.ap()[:, :, :])

        # pools that live for the whole kernel
        ppool = stack.enter_context(tc.tile_pool(name="pring", bufs=3))
        xpool = stack.enter_context(tc.tile_pool(name="xring", bufs=3))
        stp = stack.enter_context(tc.tile_pool(name="state", bufs=2))
        wk = stack.enter_context(tc.tile_pool(name="work", bufs=2))
        hop = stack.enter_context(tc.tile_pool(name="hout", bufs=3))
        psa_p = stack.enter_context(tc.tile_pool(name="ps_a", bufs=1,
                                                 space="PSUM"))
        pp_ps = stack.enter_context(tc.tile_pool(name="ps_p", bufs=1,
                                                 space="PSUM"))
        att_ps = stack.enter_context(tc.tile_pool(name="ps_t", bufs=1,
                                                  space="PSUM"))
        unt_ps = stack.enter_context(tc.tile_pool(name="ps_u", bufs=1,
                                                  space="PSUM"))

        # ---- init: G = blockpack(Aflat^T @ Wattn) ---------------------
        with tc.tile_pool(name="wa_tmp", bufs=1) as wap:
            wa_s = wap.tile([128, KC, J], bf16, name="wa_s")
            nc.sync.dma_start(out=wa_s[:, :, :], in_=wa_d.ap()[:, :, :])
            if has_bias:
                bf_s = cpool.tile([1, J], f32, name="bf_s")
                nc.sync.dma_start(out=bf_s[:, :], in_=b_d.ap()[:, :])
                bb_s = cpool.tile([1, J], bf16, name="bb_s")
                nc.vector.tensor_copy(bb_s[:, :], bf_s[:, :])
            for c in range(NCH):
                for jt in range(4):
                    jsl = slice(jt * 512, (jt + 1) * 512)
                    gps = pp_ps.tile([128, 512], f32, tag="pps", name="gps")
                    for kc in range(KC):
                        nc.tensor.matmul(
                            gps[:, :],
                            at_s[:, kc, c * 8:(c + 1) * 8, :]
                            .rearrange("p a b -> p (a b)"),
                            wa_s[:, kc, jsl],
                            start=(kc == 0), stop=(kc == KC - 1))
                    nc.vector.tensor_copy(g_sb[:, c, jsl], gps[:, :])

        # ---- persistent PSUM ------------------------------------------
        # gates: four per-bank tiles (f|o|i|g); step t writes
        # partitions 32*(t%3):+16 of each
        psaT = [psa_p.tile([128, 512], f32, tag=f"a{jt}", name="psaT")
                for jt in range(4)]
        # att bank: psz (scores colsum) alone — it holds a
        # multi-instruction accumulation group each step
        attb = att_ps.tile([128, 256], f32, tag="t", name="attb")
        psz = attb[0:1, 0:256].rearrange("p (a b) -> p a b", a=NL)
        # untranspose bank: h back in n-layout for output DMA +
        # transposed gates (all atomic writes)
        untb = unt_ps.tile([128, 776], bf16, tag="u", name="untb")
        untr = untb[0:NL, 0:512]
        pstg = untb[:, 520:776].rearrange("p (g k n) -> p g k n", g=4, k=KC)
        # softmax weight transposes get their own bank so they never
        # wait on the output flush
        pwb = unt_ps.tile([128, NCH, 2], bf16, tag="w", name="pwb")
        pstw = pwb

        pblks = {}
        xts = {}
        prod_pps = {}

        def stage_x(b):
            xt = xpool.tile([128, KC, 128], bf16, tag="xt", name="xt")
            nc.sync.dma_start(out=xt[:, :, :], in_=xs.ap()[b, :, :, :])
            xts[b] = xt

        def produce_half(b, hh, defer=False):
            """2 of the 16 matmuls of P[b] = x_block @ Wx (+b); hh=0..7.
            With defer=True the PSUM->SBUF copy is returned as a thunk so
            it can be emitted late (it has ~7 steps of slack and must not
            sit ahead of the tail's chain-critical reads in the DVE
            queue)."""
            if hh == 0:
                pblks[b] = ppool.tile([128, J], bf16, tag="pblk",
                                      name="pblk")
            jt, kp = divmod(hh, 2)
            jsl = slice(jt * 512, (jt + 1) * 512)
            if kp == 0:
                prod_pps[b] = pp_ps.tile([128, 512], f32, tag="pps",
                                         name="pps")
            pps = prod_pps[b]
            for k2 in range(2):
                kc = 2 * kp + k2
                nc.tensor.matmul(pps[:, :], xts[b][:, kc, :],
                                 wx_s[:, kc, jsl],
                                 start=(kc == 0),
                                 stop=(kc == KC - 1) and not has_bias)
            if kp == 1:
                if has_bias:
                    nc.tensor.matmul(pps[:, :], or_s[:, :], bb_s[:, jsl],
                                     start=False, stop=True)

                def copy():
                    nc.vector.tensor_copy(pblks[b][:, jsl], pps[:, :])
                if defer:
                    return copy
                copy()
            return None

        # prime the ring
        for b in range(min(2, nblk)):
            stage_x(b)
            for hh in range(2 * KC):
                produce_half(b, hh)

        cT = c0_s
        hT = h0_s
        prev = None          # (psa slice, out-row) of step t-1
        flush_row = None     # output row whose h sits in untr, not yet sent

        def tail_reads(ppsa):
            """Copy the previous step's gates out of PSUM in four
            parallel pieces (DVE + gpsimd) — the only reads of the psa
            banks, so the next step's writes unblock early."""
            rs = []
            for jt in range(4):
                r = wk.tile([NL, 512], bf16, tag=f"r{jt}", name="r")
                if jt % 2 == 0:
                    nc.vector.tensor_copy(r[:, :], ppsa[jt][:, :])
                else:
                    nc.scalar.copy(r[:, :], ppsa[jt][:, :])
                rs.append(r)
            return rs

        def tail_tr(rs, kc):
            """PE-transpose the raw gates of kc into pstg."""
            for g4 in range(4):
                nc.tensor.transpose(pstg[:, g4, kc, :],
                                    rs[g4][:, kc * 128:(kc + 1) * 128],
                                    id_s[:, :])

        def tail_chunk(ch, cT_old, cT_new, hT_new):
            """Cell update for kc pair ch in transposed layout:
            tanh on [128, ...]-shaped ops, then the c/h chain at full
            lane width.  h lands in hT_new and (via untr) the output."""
            ksl = slice(2 * ch, 2 * ch + 2)
            tfT = wk.tile([128, 4, 2, NL], bf16, tag=f"tf{ch}", name="tfT")
            nc.scalar.activation(tfT[:, :, :, :], pstg[:, :, ksl, :],
                                 AF.Tanh, scale=0.5)
            sfoi = wk.tile([128, 3, 2, NL], bf16, tag=f"sf{ch}", name="sfoi")
            nc.vector.tensor_scalar(sfoi[:, :, :, :], tfT[:, 0:3, :, :],
                                    0.5, 0.5, mult, add)
            t1 = wk.tile([128, 2, NL], f32, tag=f"t1{ch}", name="t1")
            nc.vector.tensor_tensor(t1[:, :, :], sfoi[:, 0, :, :],
                                    cT_old[:, ksl, :], mult)
            t2 = wk.tile([128, 2, NL], bf16, tag=f"t2{ch}", name="t2")
            nc.vector.tensor_tensor(t2[:, :, :], sfoi[:, 2, :, :],
                                    tfT[:, 3, :, :], mult)
            nc.vector.tensor_tensor(cT_new[:, ksl, :], t1[:, :, :],
                                    t2[:, :, :], add)
            tc_t = wk.tile([128, 2, NL], bf16, tag=f"tc{ch}", name="tc_t")
            nc.scalar.activation(tc_t[:, :, :], cT_new[:, ksl, :], AF.Tanh)
            nc.vector.tensor_tensor(hT_new[:, ksl, :], sfoi[:, 1, :, :],
                                    tc_t[:, :, :], mult)

        def untr_kc(hT_new, kc):
            nc.tensor.transpose(untr[:, kc * 128:(kc + 1) * 128],
                                hT_new[:, kc, :], idb_s[:, :])

        def flush_out(row):
            h_f = hop.tile([NL, H], bf16, tag="hf", name="h_f")
            nc.scalar.copy(h_f[:, :], untr[:, :])
            nc.sync.dma_start(out=out_d.ap()[:, row, :], in_=h_f[:, :])

        for t in range(t_steps):
            b_i, tt = divmod(t, TB)
            pblk = pblks[b_i]
            par = t % 3
            psl = slice(32 * par, 32 * par + NL)
            psa = [psaT[jt][psl, :] for jt in range(4)]

            # -- P production for block b+2 fills the step-boundary gap
            nb = b_i + 2
            prod_copy = None
            if nb < nblk:
                if tt == 0:
                    stage_x(nb)
                prod_copy = produce_half(nb, tt, defer=True)

            s2 = wk.tile([128, KC, NL, M], bf16, tag="s2", name="s2")
            if prev is not None:
                cT_new = stp.tile([128, KC, NL], f32, tag="c", name="cT_n")
                hT_new = stp.tile([128, KC, NL], bf16, tag="h", name="hT_n")
                rs = tail_reads(prev[0])
                if flush_row is not None:
                    flush_out(flush_row)
                tail_tr(rs, 0)
                tail_tr(rs, 1)
                # P inject opens the psum group; unblocks once the four
                # reads above retire
                for jt in range(4):
                    nc.tensor.matmul(psa[jt], sel_s[:, tt, :],
                                     pblk[:, jt * 512:(jt + 1) * 512],
                                     start=True, stop=False,
                                     skip_group_check=True)
                tail_tr(rs, 2)
                tail_tr(rs, 3)
                tail_chunk(0, cT, cT_new, hT_new)
                for kc in (0, 1):
                    nc.gpsimd.tensor_tensor(
                        s2[:, kc, :, :], at_s[:, kc, :, :],
                        hT_new[:, kc, :, None].broadcast_to([128, NL, M]),
                        mult)
                for jt in range(4):
                    nc.tensor.matmul(psa[jt], hT_new[:, 0, :],
                                     wh_s[:, 0, jt * 512:(jt + 1) * 512],
                                     start=False, stop=False,
                                     skip_group_check=True)
                untr_kc(hT_new, 0)
                untr_kc(hT_new, 1)
                tail_chunk(1, cT, cT_new, hT_new)
                for kc in (2, 3):
                    nc.gpsimd.tensor_tensor(
                        s2[:, kc, :, :], at_s[:, kc, :, :],
                        hT_new[:, kc, :, None].broadcast_to([128, NL, M]),
                        mult)
                for jt in range(4):
                    nc.tensor.matmul(psa[jt], hT_new[:, 1, :],
                                     wh_s[:, 1, jt * 512:(jt + 1) * 512],
                                     start=False, stop=False,
                                     skip_group_check=True)
                for kc in range(KC):
                    nc.tensor.matmul(psz[:, :, :], oc_s[:, :],
                                     s2[:, kc, :, :],
                                     start=(kc == 0), stop=(kc == KC - 1))
                for q in (2, 3):
                    for jt in range(4):
                        nc.tensor.matmul(psa[jt], hT_new[:, q, :],
                                         wh_s[:, q,
                                              jt * 512:(jt + 1) * 512],
                                         start=False, stop=False,
                                         skip_group_check=True)
                untr_kc(hT_new, 2)
                untr_kc(hT_new, 3)
                cT = cT_new
                hT = hT_new
            else:
                for jt in range(4):
                    nc.tensor.matmul(psa[jt], sel_s[:, tt, :],
                                     pblk[:, jt * 512:(jt + 1) * 512],
                                     start=True, stop=False,
                                     skip_group_check=True)
                for kc in range(KC):
                    nc.gpsimd.tensor_tensor(
                        s2[:, kc, :, :], at_s[:, kc, :, :],
                        hT[:, kc, :, None].broadcast_to([128, NL, M]),
                        mult)
                for q in range(KC):
                    for jt in range(4):
                        nc.tensor.matmul(psa[jt], hT[:, q, :],
                                         wh_s[:, q,
                                              jt * 512:(jt + 1) * 512],
                                         start=False, stop=False,
                                         skip_group_check=True)
                for kc in range(KC):
                    nc.tensor.matmul(psz[:, :, :], oc_s[:, :],
                                     s2[:, kc, :, :],
                                     start=(kc == 0), stop=(kc == KC - 1))

            # -- softmax on one partition (overlaps Wh stream on ACT/DVE)
            e_t = wk.tile([1, NL * M], bf16, tag="e", name="e_t")
            nc.scalar.activation(
                e_t[:, :], psz[:, :, :].rearrange("p a b -> p (a b)"),
                AF.Exp, scale=inv_sqrt_h)
            sum_e = wk.tile([1, NL, 1], f32, tag="sume", name="sum_e")
            nc.vector.tensor_reduce(
                sum_e[:, :, :],
                e_t[:, :].rearrange("p (a b) -> p a b", a=NL), AX, add)
            rec = wk.tile([1, NL, 1], f32, tag="rec", name="rec")
            nc.vector.reciprocal(rec[:, :, :], sum_e[:, :, :])
            w_t = wk.tile([1, NL, M], bf16, tag="wt", name="w_t")
            nc.vector.tensor_tensor(
                w_t[:, :, :],
                e_t[:, :].rearrange("p (a b) -> p a b", a=NL),
                rec[:, :, :].broadcast_to([1, NL, M]), mult)

            # -- block-diagonal softmax weights: bd[16n+m, c, q]
            for c in range(NCH):
                nc.tensor.transpose(
                    pstw[:, c, 0:1],
                    w_t[:, :, :].rearrange("p a b -> p (a b)")
                    [:, c * 128:(c + 1) * 128],
                    id_s[:1, :1])
            bd = wk.tile([128, NCH, NL], bf16, tag="bd", name="bd")
            nc.vector.tensor_tensor(
                bd[:, :, :], msk_s[:, :, :],
                pstw[:, :, 0:1].broadcast_to([128, NCH, NL]), mult)

            # -- attention gate contribution via G (K=16 per chunk)
            for jt in range(4):
                for c in range(NCH):
                    nc.tensor.matmul(psa[jt], bd[:, c, :],
                                     g_sb[:, c, jt * 512:(jt + 1) * 512],
                                     start=False, stop=(c == NCH - 1),
                                     skip_group_check=True)

            if prod_copy is not None:
                prod_copy()
            if prev is not None:
                flush_row = prev[1]
            prev = (psa, t)

        # final tail: h(T) for output row T-1
        cT_new = stp.tile([128, KC, NL], f32, tag="c", name="cT_n")
        hT_new = stp.tile([128, KC, NL], bf16, tag="h", name="hT_n")
        rs = tail_reads(prev[0])
        if flush_row is not None:
            flush_out(flush_row)
        for kc in range(KC):
            tail_tr(rs, kc)
        for ch in range(2):
            tail_chunk(ch, cT, cT_new, hT_new)
        for kc in range(KC):
            untr_kc(hT_new, kc)
        flush_out(prev[1])

    nc.compile()
    return nc


def _stage_inputs(x, A, Wx, Wh, Wattn, b, t_steps=T):
    """Shard + lay out inputs per core (host-side numpy staging)."""
    nblk = (t_steps + TB - 1) // TB
    h0 = A.mean(axis=(2, 3)).astype(np.float32)          # (N, H)
    ident = np.eye(NL, dtype=BF16)
    ones_col = np.ones((128, 1), dtype=BF16)
    ones_row = np.ones((1, 128), dtype=BF16)

    # sel[p, tt, q] = 1 iff p == 16*tt + q
    sel = np.zeros((128, TB, NL), dtype=BF16)
    for ttt in range(TB):
        for q in range(NL):
            sel[NL * ttt + q, ttt, q] = 1
    # mask3[16n+m, c, q] = 1 iff q == 8c + n
    mask3 = np.zeros((128, NCH, NL), dtype=BF16)
    for n in range(8):
        for m in range(M):
            for c in range(NCH):
                mask3[M * n + m, c, 8 * c + n] = 1

    # device gate order f|o|i|g (reference is i|f|o|g), and the g block
    # is scaled by 2 so tanh(psa/2) is right for every gate
    gperm = np.concatenate([np.arange(512, 1024), np.arange(1024, 1536),
                            np.arange(0, 512), np.arange(1536, 2048)])
    gscale = np.ones((J,), np.float32)
    gscale[1536:] = 2.0

    def wlay(w):
        w2 = w[:, gperm].astype(np.float32) * gscale
        return np.ascontiguousarray(
            w2.astype(BF16).reshape(KC, 128, J).transpose(1, 0, 2))

    wxs, whs, was = wlay(Wx), wlay(Wh), wlay(Wattn)
    bvec = np.ascontiguousarray(
        (b[gperm].astype(np.float32) * gscale).reshape(1, J))

    def tlay(v, dtype):
        return np.ascontiguousarray(
            v.T.astype(dtype).reshape(KC, 128, NL).transpose(1, 0, 2))

    maps = []
    for k in range(NCORES):
        ns = slice(k * NL, (k + 1) * NL)
        x_sh = x[ns, :t_steps].astype(BF16)              # (NL, t, D)
        # (blk, p, kc, tt*NL+n)
        xT = x_sh.transpose(2, 0, 1).reshape(KC, 128, NL, nblk, TB)
        xs_st = np.ascontiguousarray(
            xT.transpose(3, 1, 0, 4, 2).reshape(nblk, 128, KC, 128))
        A_sh = A[ns].reshape(NL, H, M).astype(BF16)
        at_st = np.ascontiguousarray(
            A_sh.transpose(1, 0, 2).reshape(KC, 128, NL, M)
            .transpose(1, 0, 2, 3))
        h0_sh = h0[ns]                                    # (NL, H)
        m = {
            "xs": xs_st, "at": at_st, "wx": wxs, "wh": whs, "wa": was,
            "h0t": tlay(h0_sh, BF16), "c0t": tlay(h0_sh, np.float32),
            "ident": ident, "identb": np.eye(128, dtype=BF16),
            "identf": np.eye(NL, dtype=np.float32),
            "ones_col": ones_col, "ones_row": ones_row,
            "sel": sel, "mask3": mask3,
        }
        if np.any(b != 0):
            m["bvec"] = bvec
        maps.append(m)
    return maps


def _get_nc(has_bias, t_steps=T):
    key = (has_bias, t_steps)
    if key not in _CACHE:
        _CACHE[key] = build(t_steps=t_steps, has_bias=has_bias)
    return _CACHE[key]


def run_cores(x, A, Wx, Wh, Wattn, b, t_steps=T, trace=False):
    from concourse.bass_utils import run_bass_kernel_spmd
    maps = _stage_inputs(x, A, Wx, Wh, Wattn, b, t_steps=t_steps)
    has_bias = "bvec" in maps[0]
    nc = _get_nc(has_bias, t_steps)
    res = run_bass_kernel_spmd(nc, maps, list(range(NCORES)), trace=trace)
    out = np.concatenate([res.results[k]["out"] for k in range(NCORES)],
                         axis=0)
    return np.asarray(out, dtype=np.float32), res


def kernel(x, A, Wx, Wh, Wattn, b):
    x = np.asarray(x, dtype=np.float32)
    A = np.asarray(A, dtype=np.float32)
    out, _ = run_cores(x, A,
                       np.asarray(Wx, dtype=np.float32),
                       np.asarray(Wh, dtype=np.float32),
                       np.asarray(Wattn, dtype=np.float32),
                       np.asarray(b, dtype=np.float32))
    return out
